# revision 49
# baseline (speedup 1.0000x reference)
"""Bahdanau-style attention kernel for Trainium2, data-parallel over batch
across 8 NeuronCores.

Reference computation (per batch b):
    W_h, W_e = W_attn[:H], W_attn[H:]
    proj   = hidden @ W_h + enc[b] @ W_e + b_attn          # [S, H]
    energy = tanh(proj)
    scores = energy @ W_v                                   # [S]
    scores = where(mask==0, -1e10, scores)
    attn   = softmax(scores)
    ctx    = attn @ enc[b]                                  # [2H]

Shapes: B=32, S=1024, H=512, 2H=1024.  8 cores x 4 batches each.

Per-core implementation (v4, mask-compacted):
  - Masked rows (mask==0) get attn weight exactly 0 and contribute nothing
    to scores or context, so the kernel only computes on the ~53% unmasked
    rows: the host assigns batches to (slot, core) sorted by unmasked count
    (so low-count slots compile to fewer compact tiles, e.g. 4/5/5/5) and
    builds per-slot gather index lists (pads point at row 0 and are
    neutralized with a -1e10 score bias).  The kernel gathers just those
    enc rows from HBM with per-tile indirect (SWDGE) DMAs at 4KB/row
    descriptors, casting f32->bf16 in flight; outputs are un-permuted on
    the host.
  - The e-contraction (enc @ W_e) needs e on partitions, so each gathered
    128x128 tile is transposed on the TensorEngine (f32r) and the PSUM
    result copied back to SBUF.
  - The bias row hb[b] = (hidden @ W_h + b_attn)[b] seeds each PSUM bank
    via a K=1 broadcast matmul (f32r); main matmuls accumulate on top.
  - Softmax without running max over the compacted scores; pads and
    (absent) masked rows underflow to exactly 0.
  - context: compact p column against gathered enc tiles (f32r ap-512
    matmuls, contraction over compact rows).
  - The full [S] attention row is reconstituted ON DEVICE with a
    selection-matrix matmul: Sel[c, s] = (idx[c] == s) built on DVE in
    fp16 (exact for integers < 2048), then attnT[1, s] = p_norm.T @ Sel.
    Masked positions never match any index and come out exactly 0.
"""

import numpy as np

B, S, H = 32, 1024, 512
E = 2 * H            # 1024
N_CORES = 8
B_LOC = B // N_CORES  # 4
ST = S // 128         # 8 s-tiles per batch (full)
NT_DEF = 5            # compacted s-tiles per batch (5*128=640 >= max 547)
# Batches are assigned to (core, slot) sorted by unmasked count, so low-count
# slots get fewer compact tiles.  For the reference seed: (4, 4, 5, 5).
NTS_DEF = (4, 5, 5, 5)
ET = E // 128         # 8 e-tiles (k-tiles of the main matmul)
KT_H = H // 128       # 4 k-tiles for hidden @ W_h

_cache = {}


def _install_tile_drain_patch():
    """walrus in this container rejects >1 sem-wait on the SP CTRL drain that
    TileContext emits at kernel tail; split the waits across 1-wait nops."""
    import concourse.tile as tile
    import concourse.mybir as mybir
    from concourse.vector_clock import ScopedClock

    if getattr(tile.TileContext, "_drain_patch_installed", False):
        return

    def _drain_and_barrier_split(self, tick_clock, wait_clock):
        nc = self.nc
        probe = nc.sync.nop(nofuse=True, hint="tail_wait_probe")
        wait_clock.add_sem_waits(
            probe.ins, ScopedClock({None: tick_clock.global_clock})
        )
        si = probe.ins.sync_info
        waits = list(si.on_wait) if si and si.on_wait else []
        if len(waits) > 1:
            si.on_wait = waits[:1]
            for w in waits[1:]:
                n = nc.sync.nop(nofuse=True, hint="tail_wait_extra")
                nsi = n.ins.sync_info
                if nsi is None:
                    n.ins.sync_info = mybir.SyncInfo(on_wait=[w], on_update=[])
                else:
                    nsi.on_wait = [w]
        nc.sync.drain()
        nc.all_engine_barrier()
        assert self.sems is not None
        popped = nc._tile_sem_poison_stack.pop()
        assert popped is self._sem_poison
        # chunked clear_and_free_semaphores: walrus rejects RANGE_CLEAR ISA
        # instructions spanning more than a few semaphores ("ISA wrong
        # length"), so clear in <=3-wide ranges.
        sems = list(self.sems.allocated().values())
        sem_nums = sorted(s.num if hasattr(s, "num") else s for s in sems)
        if sem_nums:
            runs = []
            lo = prev = sem_nums[0]
            for n in sem_nums[1:]:
                if n == prev + 1:
                    prev = n
                else:
                    runs.append((lo, prev))
                    lo = prev = n
            runs.append((lo, prev))
            for lo, hi in runs:
                for c0 in range(lo, hi + 1, 3):
                    c1 = min(c0 + 2, hi)
                    r = range(c0, c1 + 1)
                    assert nc._state.free_isdisjoint(r)
                    nc.gpsimd.dma_reset(r)
                    nc.gpsimd.sem_clear(r)
            nc._state.prepend_free_semaphores(sem_nums)
            for poison_set in nc._tile_sem_poison_stack:
                poison_set.update(sem_nums)
        nc.all_engine_barrier()

    tile.TileContext._drain_and_barrier = _drain_and_barrier_split
    tile.TileContext._drain_patch_installed = True


def _split_multiwaits(nc, max_waits=1):
    """walrus's setupSyncWait rejects instructions carrying more than a couple
    of semaphore waits.  Move excess waits onto same-engine nops inserted
    immediately before the offending instruction (engine executes in order, so
    semantics are identical)."""
    import concourse.mybir as mybir

    for f in nc.m.functions:
        for bb in f.blocks:
            out = []
            for inst in bb.instructions:
                si = inst.sync_info
                waits = list(si.on_wait) if si and si.on_wait else []
                lim = max_waits
                if len(waits) > lim:
                    excess = waits[:-lim]
                    si.on_wait = waits[-lim:]
                    for i in range(0, len(excess), max_waits):
                        nop = mybir.InstNoOp(
                            name=f"I-{nc.next_id()}-waitsplit", ins=[], outs=[]
                        )
                        nop.engine = inst.engine
                        nop.sync_info = mybir.SyncInfo(
                            on_wait=excess[i:i + max_waits], on_update=[]
                        )
                        nc.register_instruction(nop, overwrite=True)
                        out.append(nop)
                out.append(inst)
            bb.instructions[:] = out


def _emit_context_pair(nc, pctx_pool, ctxpool, pends, ctx_d):
    """Deferred context emission: ctx(b) = (p_r @ enc_b) / denom for each
    pending batch.  The two E-halves accumulate sequentially in PSUM banks
    (f32r matmuls must write partition 0)."""
    import concourse.mybir as mybir
    f32 = mybir.dt.float32
    for i, (b, enc_b, p_r, rd, nt) in enumerate(pends):
        ctx_sb = ctxpool.tile([1, E], f32, tag="ctx_sb")
        for h2 in range(2):
            p_c = pctx_pool.tile([1, 512], f32, tag="p_c")
            for st in range(nt):
                nc.tensor.matmul(
                    p_c[:], p_r[:, st:st + 1],
                    enc_b[:, st * E + 512 * h2: st * E + 512 * (h2 + 1)],
                    start=(st == 0), stop=(st == nt - 1),
                )
            nc.vector.tensor_scalar_mul(
                ctx_sb[:, 512 * h2:512 * (h2 + 1)], p_c[:], rd[:])
        nc.sync.dma_start(ctx_d[b][None, :], ctx_sb[:])


def build_kernel(n_iters: int = 1, nts: tuple = NTS_DEF):
    """Build the per-core Bass program.  n_iters>1 repeats the whole compute
    body (for slope-based wall-clock timing); outputs are just rewritten."""
    _install_tile_drain_patch()
    import concourse.bass as bass
    import concourse.tile as tile
    import concourse.mybir as mybir
    from concourse.mybir import AluOpType as alu
    from concourse.mybir import ActivationFunctionType as act

    f32 = mybir.dt.float32
    f32r = mybir.dt.float32r
    f16 = mybir.dt.float16
    bf16 = mybir.dt.bfloat16
    i32 = mybir.dt.int32
    ntmax = max(nts)
    NC = ntmax * 128  # compact-row capacity per batch slot

    nc = bass.Bass("TRN2", target_bir_lowering=False, debug=False,
                   num_devices=N_CORES)

    hidden_d = nc.dram_tensor("hidden", [B_LOC, H], f32r, kind="ExternalInput").ap()
    enc_d = nc.dram_tensor("enc", [B_LOC * S, E], f32r, kind="ExternalInput").ap()
    idxg_d = nc.dram_tensor("idx_g", [B_LOC, NC], i32, kind="ExternalInput").ap()
    idxs_d = nc.dram_tensor("idx_s", [B_LOC, NC], f32, kind="ExternalInput").ap()
    wattn_d = nc.dram_tensor("w_attn", [3 * H, H], f32r, kind="ExternalInput").ap()
    battn_d = nc.dram_tensor("b_attn", [H], f32r, kind="ExternalInput").ap()
    wv_d = nc.dram_tensor("w_v", [H], f32, kind="ExternalInput").ap()
    ctx_d = nc.dram_tensor("out_ctx", [B_LOC, E], f32, kind="ExternalOutput").ap()
    attn_d = nc.dram_tensor("out_attn", [B_LOC, S], f32, kind="ExternalOutput").ap()

    with tile.TileContext(nc) as tc:
        with (
            tc.tile_pool(name="const", bufs=1) as cpool,
            tc.tile_pool(name="enc", bufs=3) as encpool,
            tc.tile_pool(name="encT", bufs=7) as encTpool,
            tc.tile_pool(name="work", bufs=4) as wpool,
            tc.tile_pool(name="perb", bufs=3) as bpool,
            tc.tile_pool(name="ctxp", bufs=2) as ctxpool,
            tc.tile_pool(name="ptr", bufs=2, space="PSUM") as ptr_pool,
            tc.tile_pool(name="pproj", bufs=3, space="PSUM") as pproj_pool,
            tc.tile_pool(name="pctx", bufs=1, space="PSUM") as pctx_pool,
            tc.tile_pool(name="psmall", bufs=1, space="PSUM") as psmall_pool,
        ):
            # ---------------- constants / weights ----------------
            ones_f = cpool.tile([128, 128], f32)
            nc.vector.memset(ones_f[:], 1.0)
            ones_col_f = cpool.tile([128, 1], f32)
            nc.vector.memset(ones_col_f[:], 1.0)
            ones_row_f = cpool.tile([1, 128], f32)
            nc.vector.memset(ones_row_f[:], 1.0)

            id_f = cpool.tile([128, 128], f32)
            nc.gpsimd.affine_select(
                id_f[:], ones_f[:], pattern=[[1, 128]],
                compare_op=alu.is_equal, fill=0.0, base=0,
                channel_multiplier=-1,
            )
            id_sb = cpool.tile([128, 128], bf16)
            nc.vector.tensor_copy(id_sb[:], id_f[:])

            # W_e: rows H..3H of W_attn -> [128, ET*512]
            w_e = cpool.tile([128, ET * H], bf16)
            nc.gpsimd.dma_start(
                w_e[:].rearrange("p (kt h) -> p kt h", h=H),
                wattn_d[H:3 * H, :].rearrange("(kt p) h -> p kt h", p=128),
            )
            # W_h: rows 0..H -> [128, KT_H*512]
            w_h = cpool.tile([128, KT_H * H], f32r)
            nc.sync.dma_start(
                w_h[:].rearrange("p (kt h) -> p kt h", h=H),
                wattn_d[0:H, :].rearrange("(kt p) h -> p kt h", p=128),
            )
            # hidden transposed: [128, KT_H*B_LOC]
            hidT = cpool.tile([128, KT_H * B_LOC], f32r)
            for kt in range(KT_H):
                nc.sync.dma_start(
                    hidT[:, kt * B_LOC:(kt + 1) * B_LOC],
                    hidden_d[:, kt * 128:(kt + 1) * 128].rearrange("b p -> p b"),
                )
            wv_row = cpool.tile([1, H], f32)
            nc.sync.dma_start(wv_row[:], wv_d[None, :])
            battn_row = cpool.tile([1, H], f32r)
            nc.sync.dma_start(battn_row[:], battn_d[None, :])

            # broadcast W_v to all partitions
            p_wv = psmall_pool.tile([128, H], f32, tag="small")
            nc.tensor.matmul(p_wv[:], ones_row_f[:], wv_row[:],
                             start=True, stop=True)
            wv_bc = cpool.tile([128, H], f32)
            nc.any.tensor_copy(wv_bc[:], p_wv[:])

            # hb = hidden @ W_h + b_attn  (rows 0..3 of PSUM)
            ones_tl = cpool.tile([1, B_LOC], f32r)
            nc.vector.tensor_copy(ones_tl[:], ones_f[0:1, 0:B_LOC])
            p_ph = psmall_pool.tile([B_LOC, H], f32, tag="small")
            for kt in range(KT_H):
                nc.tensor.matmul(
                    p_ph[:],
                    hidT[:, kt * B_LOC:(kt + 1) * B_LOC],
                    w_h[:, kt * H:(kt + 1) * H],
                    start=(kt == 0), stop=False,
                )
            nc.tensor.matmul(p_ph[:], ones_tl[:],
                             battn_row[:], start=False, stop=True)
            hb = cpool.tile([B_LOC, H], f32r)
            nc.any.tensor_copy(hb[:], p_ph[:])
            # all four hb rows gathered onto partition 0 (matmul rhs needs
            # base partition 0); SBUF->SBUF DMA, 4 descriptors, setup-only.
            hb_flat = cpool.tile([1, B_LOC * H], f32r)
            nc.sync.dma_start(
                hb_flat[:].rearrange("p (b h) -> p b h", h=H), hb[:]
            )
            ones_row_r = cpool.tile([1, 128], f32r)
            nc.vector.tensor_copy(ones_row_r[:], ones_f[0:1, :])

            # gather indices: int32 for the indirect DMA, f32/f16 for the
            # pad bias and the attn selection matrix.
            idxg_sb = cpool.tile([128, B_LOC * ntmax], i32)
            idxs_sb = cpool.tile([128, B_LOC * ntmax], f32)
            for b in range(B_LOC):
                nc.sync.dma_start(
                    idxg_sb[:, b * ntmax:(b + 1) * ntmax],
                    idxg_d[b].rearrange("(t p) -> p t", p=128),
                )
                nc.sync.dma_start(
                    idxs_sb[:, b * ntmax:(b + 1) * ntmax],
                    idxs_d[b].rearrange("(t p) -> p t", p=128),
                )
            idxs16 = cpool.tile([128, B_LOC * ntmax], f16)
            nc.vector.tensor_copy(idxs16[:], idxs_sb[:])
            # pad bias: -1e10 where idx_s < 0 (pads), else 0
            mbias = cpool.tile([128, B_LOC * ntmax], f32)
            nc.vector.tensor_scalar(
                mbias[:], idxs_sb[:], 0.0, 1e10,
                op0=alu.is_ge, op1=alu.mult,
            )
            nc.vector.tensor_scalar(
                mbias[:], mbias[:], 1.0, -1e10,
                op0=alu.mult, op1=alu.add,
            )
            # iota over s (same for every partition), fp16-exact
            iota_i = cpool.tile([128, S], i32)
            nc.gpsimd.iota(iota_i[:], pattern=[[1, S]], base=0,
                           channel_multiplier=0)
            iota16 = cpool.tile([128, S], f16)
            nc.vector.tensor_copy(iota16[:], iota_i[:])

            # ---------------- main loop ----------------
            for it in range(n_iters):
                pend = []  # [(b, enc_b, p_r, rd)] awaiting context emission
                for b in range(B_LOC):
                    nt = nts[b]
                    enc_b = encpool.tile([128, ntmax * E], bf16, tag="enc_b")
                    # one indirect (SWDGE) gather per compact tile: the DMA
                    # applies one row-offset per partition and casts f32->bf16
                    # in flight (halves SBUF, 1.0 cyc/row PE transposes).
                    for t in range(nt):
                        nc.gpsimd.indirect_dma_start(
                            out=enc_b[:, t * E:(t + 1) * E],
                            out_offset=None,
                            in_=enc_d[:, :],
                            in_offset=bass.IndirectOffsetOnAxis(
                                ap=idxg_sb[:, b * ntmax + t:b * ntmax + t + 1],
                                axis=0,
                            ),
                        )

                    s_sb = bpool.tile([128, ntmax], f32, tag="s_sb")
                    # Two-stage software pipeline over compact tiles: the
                    # transposes for tile st+1 are emitted BEFORE the proj
                    # matmuls of tile st.
                    encT_q = []
                    for st in range(nt + 1):
                        if st < nt:
                            encT = encTpool.tile([128, E], bf16, tag="encT")
                            for g in range(2):
                                p_tr = ptr_pool.tile([128, 512], bf16, tag="p_tr")
                                for j4 in range(4):
                                    j = g * 4 + j4
                                    nc.tensor.transpose(
                                        p_tr[:, j4 * 128:(j4 + 1) * 128],
                                        enc_b[:, st * E + j * 128: st * E + (j + 1) * 128],
                                        id_sb[:],
                                    )
                                nc.any.tensor_copy(
                                    encT[:, g * 512:(g + 1) * 512], p_tr[:]
                                )
                            encT_q.append(encT)
                        if st == 0:
                            continue
                        stp = st - 1
                        encT_p = encT_q.pop(0)
                        # K=1 broadcast matmul seeds the bank with hb[b];
                        # main matmuls accumulate on top.
                        p_proj = pproj_pool.tile([128, H], f32, tag="p_proj")
                        nc.tensor.matmul(
                            p_proj[:], ones_row_r[:],
                            hb_flat[:, b * H:(b + 1) * H],
                            start=True, stop=False,
                        )
                        for j in range(ET):
                            nc.tensor.matmul(
                                p_proj[:],
                                encT_p[:, j * 128:(j + 1) * 128],
                                w_e[:, j * H:(j + 1) * H],
                                start=False, stop=(j == ET - 1),
                                skip_group_check=True,
                            )
                        energy = wpool.tile([128, H], f32, tag="energy")
                        nc.scalar.activation(energy[:], p_proj[:], act.Tanh)
                        scr = wpool.tile([128, H], f32, tag="scr")
                        nc.vector.tensor_mul(scr[:], energy[:], wv_bc[:])
                        nc.vector.reduce_sum(
                            s_sb[:, stp:stp + 1], scr[:], axis=mybir.AxisListType.X
                        )
                        if stp == 2 and pend:
                            _emit_context_pair(nc, pctx_pool, ctxpool, pend,
                                               ctx_d)
                            pend = []

                    # ---- softmax over the compact rows of batch b ----
                    sm = bpool.tile([128, ntmax], f32, tag="sm")
                    nc.vector.tensor_add(
                        sm[:, :nt], s_sb[:, :nt],
                        mbias[:, b * ntmax:b * ntmax + nt]
                    )
                    p_exp = bpool.tile([128, ntmax], f32, tag="p_exp")
                    rowsum = bpool.tile([128, 1], f32, tag="rowsum")
                    nc.scalar.activation(p_exp[:, :nt], sm[:, :nt], act.Exp,
                                         accum_out=rowsum[:])
                    # bf16 copy of p_exp for the context matmul
                    p_r = bpool.tile([128, ntmax], bf16, tag="p_r")
                    nc.any.tensor_copy(p_r[:, :nt], p_exp[:, :nt])
                    p_den = psmall_pool.tile([1, 1], f32, tag="small")
                    nc.tensor.matmul(p_den[:], rowsum[:], ones_col_f[:],
                                     start=True, stop=True)
                    rd = bpool.tile([1, 1], f32, tag="rd")
                    nc.vector.reciprocal(rd[:], p_den[:])
                    # unnormalized p in fp16: the attn matmuls don't wait on
                    # the reciprocal chain; 1/denom is folded into the Act
                    # output copy's scale below.
                    p_n16 = bpool.tile([128, ntmax], f16, tag="p_n16")
                    nc.vector.tensor_copy(p_n16[:, :nt], p_exp[:, :nt])

                    # ---- scatter attn back to the full [S] row ----
                    # Sel[c, s] = (idx[c] == s), fp16-exact; attnT = p_n.T @ Sel
                    attn_row = bpool.tile([1, S], f32, tag="attn_row")
                    p_at = [pctx_pool.tile([1, 512], f32, tag="p_c",
                                           name=f"p_at{_sh}")
                            for _sh in range(2)]
                    for ct in range(nt):
                        sel_t = wpool.tile([128, S], f16, tag="sel_t")
                        nc.vector.tensor_tensor(
                            out=sel_t[:],
                            in0=idxs16[:, b * ntmax + ct:b * ntmax + ct + 1
                                       ].to_broadcast([128, S]),
                            in1=iota16[:],
                            op=alu.is_equal,
                        )
                        for sh in range(2):
                            nc.tensor.matmul(
                                p_at[sh][:], p_n16[:, ct:ct + 1],
                                sel_t[:, sh * 512:(sh + 1) * 512],
                                start=(ct == 0), stop=(ct == nt - 1),
                            )
                    for sh in range(2):
                        nc.scalar.activation(
                            attn_row[:, sh * 512:(sh + 1) * 512],
                            p_at[sh][:], act.Copy, scale=rd[:])
                    nc.sync.dma_start(attn_d[b][None, :], attn_row[:])
                    pend.append((b, enc_b, p_r, rd, nt))
                _emit_context_pair(nc, pctx_pool, ctxpool, pend, ctx_d)
                pend = []

    _split_multiwaits(nc)
    return nc


def _get_nc(n_iters: int = 1, nts: tuple = NTS_DEF):
    key = ("nc", n_iters, nts)
    if key not in _cache:
        _cache[key] = build_kernel(n_iters, nts)
    return _cache[key]


def default_nts(mask):
    """Per-slot tile counts for this mask (for external builders)."""
    return _plan(np.ascontiguousarray(np.asarray(mask, dtype=np.int32)))[1]


def _plan(mask):
    """Assign batches to (slot, core) sorted ascending by unmasked count so
    low-count slots get fewer compact tiles.  Returns (perm [B_LOC, N_CORES]
    original-batch index per (slot, core), nts per-slot tile counts)."""
    counts = (mask != 0).sum(axis=1)
    order = np.argsort(counts, kind="stable")
    perm = order.reshape(B_LOC, N_CORES)
    nts = tuple(int(-(-int(counts[perm[s]].max()) // 128))
                for s in range(B_LOC))
    return perm, nts


def _build_indices(mask_rows, ntmax):
    """Indices of unmasked rows per slot, padded to ntmax*128.
    Returns (idx_g int32 [B_LOC, ntmax*128] rows into the per-core flat enc,
             idx_s f32 [B_LOC, ntmax*128] local row or -1 for pads)."""
    cap = ntmax * 128
    idx_g = np.zeros((B_LOC, cap), np.int32)
    idx_s = np.full((B_LOC, cap), -1.0, np.float32)
    for sl in range(B_LOC):
        rows = np.nonzero(mask_rows[sl] != 0)[0].astype(np.int32)
        n = len(rows)
        if n > cap:
            raise ValueError(f"mask count {n} exceeds capacity {cap}")
        idx_g[sl, :n] = sl * S + rows
        idx_g[sl, n:] = sl * S  # pads: any valid row; bias kills them
        idx_s[sl, :n] = rows.astype(np.float32)
    return idx_g, idx_s


def shard_inputs(hidden, encoder_outputs, mask, W_attn, b_attn, W_v,
                 nts: tuple = None):
    hidden = np.ascontiguousarray(np.asarray(hidden, dtype=np.float32))
    enc = np.ascontiguousarray(np.asarray(encoder_outputs, dtype=np.float32))
    mask = np.ascontiguousarray(np.asarray(mask, dtype=np.int32))
    W_attn = np.ascontiguousarray(np.asarray(W_attn, dtype=np.float32))
    b_attn = np.ascontiguousarray(np.asarray(b_attn, dtype=np.float32))
    W_v = np.ascontiguousarray(np.asarray(W_v, dtype=np.float32))
    perm, real_nts = _plan(mask)
    if nts is None:
        nts = real_nts
    assert all(r <= n for r, n in zip(real_nts, nts)), (real_nts, nts)
    ntmax = max(nts)
    in_maps = []
    for c in range(N_CORES):
        gs = perm[:, c]          # original batch per slot on this core
        idx_g, idx_s = _build_indices(mask[gs], ntmax)
        in_maps.append({
            "hidden": hidden[gs],
            "enc": enc[gs].reshape(B_LOC * S, E),
            "idx_g": idx_g,
            "idx_s": idx_s,
            "w_attn": W_attn,
            "b_attn": b_attn,
            "w_v": W_v,
        })
    return in_maps


def kernel(hidden, encoder_outputs, mask, W_attn, b_attn, W_v):
    from concourse.bass_utils import run_bass_kernel_spmd

    mask_np = np.ascontiguousarray(np.asarray(mask, dtype=np.int32))
    perm, real_nts = _plan(mask_np)
    nts = real_nts
    nc = _get_nc(1, nts)
    in_maps = shard_inputs(hidden, encoder_outputs, mask_np, W_attn, b_attn,
                           W_v, nts)
    res = run_bass_kernel_spmd(nc, in_maps, list(range(N_CORES)))
    context = np.empty((B, E), np.float32)
    attn_w = np.empty((B, S), np.float32)
    for sl in range(B_LOC):
        for c in range(N_CORES):
            g = int(perm[sl, c])
            context[g] = res.results[c]["out_ctx"][sl]
            attn_w[g] = res.results[c]["out_attn"][sl]
    return context, attn_w
